# revision 20
# baseline (speedup 1.0000x reference)
"""EulerRotaryAttention Trainium2 kernel (bf16 matmul pipeline).

Sharding: 8 cores = 2 (batch) x 4 (head groups of 4 heads).  Each core
computes the qkv projection for its heads, rotary attention, and a partial
o-projection; the host sums partials over the 4 head groups per batch.

Device dataflow (zero on-device transposes):
  - x^T arrives pre-transposed from the host as (d, n), bf16.
  - Q^T, K^T computed directly in (feat, tok) layout with the projection
    weights as the stationary matmul operand; fp32 PSUM accumulation.
  - RoPE rotation applied during PSUM eviction.  Features are
    host-permuted (de-interleaved) so rotation pairs sit 32 partitions
    apart; cos/sin tables host-precomputed (replicating the reference
    fp32 arithmetic).  PSUM->bf16 cast on ScalarE, swap-half copies and
    multiply/add on VectorE in bf16 fast modes.
  - S^T in (k, q) layout (k on partitions, q free), causal tiles only;
    matmul streams are clipped to the causal column range per PSUM bank.
  - exp on ScalarE (scores ~ N(0,1): no max subtraction needed) into one
    (128, kt, 1024) bf16 tile per (head, q-chunk); the 8 diagonal 128x128
    subtiles are masked with a single strided tensor_tensor against a
    replicated 0/1 triangle.
  - PV: lhsT = [V | 1] (bf16) so the fp32 PSUM accumulator yields both
    A^T (feats on partitions, q free) and the softmax denominators.
  - denominators: batched VectorE reciprocal (no Ln -> only the Exp ACT
    table is ever loaded); GpSimd broadcast across partitions pairs two
    heads into one (128, 1024) scale tile; one in-place multiply
    normalizes each A^T head pair.
  - o-projection consumes A^T directly as lhsT; the partial (n, d) fp32
    output is written per core and summed on the host.
"""

import math

import numpy as np

B, N, D, H = 2, 2048, 1024, 16
DH = D // H  # 64
HL = 4  # local heads per core
DL = HL * DH  # 256 local features
KC = D // 128  # 8 contraction chunks
NT = N // 128  # 16 token tiles
NCH = N // 1024  # 2 wide column chunks
NCORES = 8

EULER_BASIS = (1.0, math.pi, math.e, math.pi * math.e, math.pi / math.e)

_PROG = None
LAST_RESULTS = None


def _build_program():
    import concourse.bass as bass
    import concourse.mybir as mybir
    import concourse.tile as tile
    from concourse import bacc

    f32 = mybir.dt.float32
    bf = mybir.dt.bfloat16
    AF = mybir.ActivationFunctionType

    nc = bacc.Bacc("TRN2", target_bir_lowering=False, num_devices=NCORES)

    xT = nc.declare_dram_parameter("xT", [128, KC, N], bf, isOutput=False)
    wq = nc.declare_dram_parameter("wq", [128, KC, DL], bf, isOutput=False)
    wk = nc.declare_dram_parameter("wk", [128, KC, DL], bf, isOutput=False)
    wv = nc.declare_dram_parameter("wv", [128, KC, DL], bf, isOutput=False)
    wo = nc.declare_dram_parameter("wo", [128, 2, D], bf, isOutput=False)
    ctab = nc.declare_dram_parameter("ctab", [128, 2, N], bf, isOutput=False)
    stab = nc.declare_dram_parameter("stab", [128, 2, N], bf, isOutput=False)
    tri8 = nc.declare_dram_parameter("tri8", [128, 8, 128], bf, isOutput=False)
    o_out = nc.declare_dram_parameter("o_out", [NT, 128, D], f32, isOutput=True)

    with tile.TileContext(nc) as tc:
        with tc.tile_pool(name="persist", bufs=1) as persist:
            # rotated Q^T / K^T: (256 feats, N) as 2 x (128, N), bf16
            qt_rot = [
                persist.tile([128, N], bf, tag=f"qt{m}", name=f"qt{m}")
                for m in range(2)
            ]
            kt_rot = [
                persist.tile([128, N], bf, tag=f"kt{m}", name=f"kt{m}")
                for m in range(2)
            ]
            # V for all heads with appended ones column: (128, NT, HL, 65)
            vones = persist.tile([128, NT, HL, DH + 1], bf, tag="vones", name="vones")
            nc.vector.memset(vones[:, :, :, DH : DH + 1], 1.0)
            # A^T head pairs: (128, N) bf16
            at2 = [
                persist.tile([128, N], bf, tag=f"at{m}", name=f"at{m}")
                for m in range(2)
            ]

            # ================= phase 1: projections =================
            with (
                tc.tile_pool(name="p1c", bufs=1) as p1c,
                tc.tile_pool(name="rot_tmp", bufs=3) as rot_tmp,
                tc.tile_pool(name="psum_qkt", bufs=2, space="PSUM") as psum_qkt,
                tc.tile_pool(name="psum_v", bufs=2, space="PSUM") as psum_v,
            ):
                wq_sb = p1c.tile([128, KC, DL], bf, tag="wq")
                wk_sb = p1c.tile([128, KC, DL], bf, tag="wk")
                wv_sb = p1c.tile([128, KC, DL], bf, tag="wv")
                ctab_sb = p1c.tile([128, 2, N], bf, tag="ctab")
                stab_sb = p1c.tile([128, 2, N], bf, tag="stab")
                xT_sb = p1c.tile([128, KC, N], bf, tag="xT")
                # order matters: the first QKT matmuls need wq + early xT
                # chunks; tables are only needed at the first eviction
                nc.sync.dma_start(out=wq_sb[:], in_=wq[:])
                for kc in range(KC):
                    nc.sync.dma_start(out=xT_sb[:, kc, :], in_=xT[:, kc, :])
                nc.sync.dma_start(out=wk_sb[:], in_=wk[:])
                nc.sync.dma_start(out=wv_sb[:], in_=wv[:])
                nc.sync.dma_start(out=ctab_sb[:], in_=ctab[:])
                nc.sync.dma_start(out=stab_sb[:], in_=stab[:])

                # Q^T / K^T: lhsT = w[kc, feats], rhs = xT[kc, toks]
                for w_sb, rot in ((wq_sb, qt_rot), (wk_sb, kt_rot)):
                    for mt in range(2):
                        for nh in range(NCH):  # 1024-wide tok chunks
                            nsl = slice(nh * 1024, (nh + 1) * 1024)
                            psum = psum_qkt.tile([128, 1024], f32, tag="qkt")
                            for kc in range(KC):
                                for nq in range(2):
                                    nc.tensor.matmul(
                                        psum[:, nq * 512 : (nq + 1) * 512],
                                        w_sb[:, kc, mt * 128 : (mt + 1) * 128],
                                        xT_sb[
                                            :,
                                            kc,
                                            nh * 1024
                                            + nq * 512 : nh * 1024
                                            + (nq + 1) * 512,
                                        ],
                                        start=(kc == 0),
                                        stop=(kc == KC - 1),
                                    )
                            # rotation eviction:
                            #   rot = raw * ctab + swap32(raw) * stab
                            raw = rot_tmp.tile([128, 1024], bf, tag="raw", name="raw")
                            nc.vector.tensor_copy(out=raw[:], in_=psum[:])
                            nc.vector.tensor_mul(
                                rot[mt][:, nsl], raw[:], ctab_sb[:, mt, nsl]
                            )
                            raws = rot_tmp.tile([128, 1024], bf, tag="rs", name="raws")
                            for g in range(4):
                                s = g ^ 1
                                nc.scalar.copy(
                                    out=raws[g * 32 : (g + 1) * 32, :],
                                    in_=raw[s * 32 : (s + 1) * 32, :],
                                )
                            tmp = rot_tmp.tile([128, 1024], bf, tag="rt", name="tmp")
                            nc.vector.tensor_mul(tmp[:], raws[:], stab_sb[:, mt, nsl])
                            nc.vector.tensor_add(
                                rot[mt][:, nsl], rot[mt][:, nsl], tmp[:]
                            )

                # V: lhsT = xT[kc, toks], rhs = wv[kc, feats]
                for tt in range(NT):
                    vpsum = psum_v.tile([128, DL], f32, tag="v")
                    for kc in range(KC):
                        nc.tensor.matmul(
                            vpsum[:],
                            xT_sb[:, kc, tt * 128 : (tt + 1) * 128],
                            wv_sb[:, kc, :],
                            start=(kc == 0),
                            stop=(kc == KC - 1),
                        )
                    # single strided eviction for all 4 heads of this tile
                    nc.scalar.copy(
                        out=vones[:, tt, :, 0:DH],
                        in_=vpsum[:].rearrange("p (h d) -> p h d", h=HL),
                    )

            # ============ phase 2a: attention ============
            with (
                tc.tile_pool(name="p2c", bufs=1) as p2c,
                tc.tile_pool(name="exps_pool", bufs=2) as exps_pool,
                tc.tile_pool(name="norm_pool", bufs=2) as norm_pool,
                tc.tile_pool(name="bcast_pool", bufs=2) as bcast_pool,
                tc.tile_pool(name="dscr_pool", bufs=4, space="DRAM") as dscr_pool,
                tc.tile_pool(name="ostage_pool", bufs=3) as ostage_pool,
                tc.tile_pool(name="psum_s", bufs=2, space="PSUM") as psum_s,
                tc.tile_pool(name="psum_pv", bufs=1, space="PSUM") as psum_pv,
                tc.tile_pool(name="psum_o", bufs=1, space="PSUM") as psum_o,
            ):
                tri8_sb = p2c.tile([128, 8, 128], bf, tag="tri8")
                wo_sb = p2c.tile([128, 2, D], bf, tag="wo")
                nc.sync.dma_start(out=tri8_sb[:], in_=tri8[:])
                nc.sync.dma_start(out=wo_sb[:], in_=wo[:])

                def o_proj_block(qc):
                    # o-projection for the 8 token tiles whose A^T columns
                    # were normalized by q-chunk qc
                    for tt in range(8 * qc, 8 * qc + 8):
                        opsum = psum_o.tile([128, D], f32, tag="o", name="opsum")
                        for hp in range(2):
                            for nb in range(2):
                                nc.tensor.matmul(
                                    opsum[:, nb * 512 : (nb + 1) * 512],
                                    at2[hp][:, tt * 128 : (tt + 1) * 128],
                                    wo_sb[:, hp, nb * 512 : (nb + 1) * 512],
                                    start=(hp == 0),
                                    stop=(hp == 1),
                                )
                        ost = ostage_pool.tile([128, D], f32, tag="ost", name="ost")
                        nc.scalar.copy(out=ost[:, 0:512], in_=opsum[:, 0:512])
                        nc.vector.tensor_copy(out=ost[:, 512:D], in_=opsum[:, 512:D])
                        nc.sync.dma_start(out=o_out[tt], in_=ost[:])

                for qch in range(NCH):
                    qsl = slice(qch * 1024, (qch + 1) * 1024)
                    nkt = 8 * qch + 8
                    # denominator rows live at partitions 0/32/64/96 (the
                    # only legal engine start partitions); unused rows are
                    # memset to 1.0 so the batched reciprocal stays finite
                    dnm4 = norm_pool.tile([97, 1024], f32, tag="dnm", name="dnm4")
                    rcp4 = norm_pool.tile([97, 1024], f32, tag="rcp", name="rcp4")
                    nc.gpsimd.memset(dnm4[:], 1.0)
                    for h in range(HL):
                        mt, roff = h // 2, (h % 2) * 64
                        if qch == 1 and h == 1:
                            # previous q-chunk's A^T is normalized by now;
                            # its o-projection overlaps this chunk's attention
                            o_proj_block(0)
                        exps = exps_pool.tile([128, NT, 1024], bf, tag="e", name="exps")
                        for kt in range(nkt):
                            j = kt - 8 * qch
                            jo = max(j, 0) * 128
                            spsum = psum_s.tile([128, 1024], f32, tag="s", name="spsum")
                            for nq in range(2):
                                lo = max(jo, nq * 512)
                                hi = (nq + 1) * 512
                                if lo >= hi:
                                    continue
                                nc.tensor.matmul(
                                    spsum[:, lo:hi],
                                    kt_rot[mt][
                                        roff : roff + 64, kt * 128 : (kt + 1) * 128
                                    ],
                                    qt_rot[mt][
                                        roff : roff + 64,
                                        qch * 1024 + lo : qch * 1024 + hi,
                                    ],
                                    start=True,
                                    stop=True,
                                )
                            nc.scalar.activation(
                                exps[:, kt, jo:1024], spsum[:, jo:1024], AF.Exp
                            )
                        # mask all 8 diagonal 128x128 subtiles in one op:
                        # element (p, j, c) -> exps[p, 8*qch + j, j*128 + c]
                        sub = exps[:, 8 * qch, :]
                        diag = bass.AP(
                            tensor=sub.tensor,
                            offset=sub.offset,
                            ap=[list(sub.ap[0]), [1152, 8], [1, 128]],
                        )
                        nc.vector.tensor_mul(diag, diag, tri8_sb[:])
                        # PV accumulation as one uninterrupted group
                        pv = psum_pv.tile([DH + 1, 1024], f32, tag="pv", name="pv")
                        # last kt contributing to each 512-col bank
                        last_kt = (8 * qch + 3, 8 * qch + 7)
                        for kt in range(nkt):
                            j = kt - 8 * qch
                            jo = max(j, 0) * 128
                            for nq in range(2):
                                lo = max(jo, nq * 512)
                                hi = (nq + 1) * 512
                                if lo >= hi:
                                    continue
                                nc.tensor.matmul(
                                    pv[:, lo:hi],
                                    vones[:, kt, h, :],
                                    exps[:, kt, lo:hi],
                                    start=(kt == 0),
                                    stop=(kt == last_kt[nq]),
                                )
                        # stash denominator, evict unnormalized A^T
                        nc.scalar.copy(
                            out=dnm4[32 * h : 32 * h + 1, :],
                            in_=pv[DH : DH + 1, :],
                        )
                        nc.vector.tensor_copy(
                            out=at2[mt][roff : roff + DH, qsl], in_=pv[0:DH, :]
                        )
                    # batched reciprocal of the 4 denominators
                    nc.vector.reciprocal(rcp4[:], dnm4[:])
                    for mt in range(2):
                        bc = bcast_pool.tile([128, 1024], f32, tag="bc", name="bc")
                        # broadcast each head's reciprocal row across 64
                        # partitions: bounce through DRAM, then a step-0
                        # partition DMA (legal for DRAM sources only; POOL's
                        # partition_broadcast ignores non-zero base
                        # partitions on hardware)
                        for half in range(2):
                            row = rcp4[64 * mt + 32 * half : 64 * mt + 32 * half + 1, :]
                            rdram = dscr_pool.tile([1, 1024], f32, tag="rd", name="rd")
                            nc.sync.dma_start(out=rdram[:], in_=row)
                            rd = rdram[:]
                            nc.sync.dma_start(
                                out=bc[64 * half : 64 * half + 64, :],
                                in_=bass.AP(
                                    tensor=rd.tensor,
                                    offset=rd.offset,
                                    ap=[[0, 64], [1, 1024]],
                                ),
                            )
                        nc.vector.tensor_mul(at2[mt][:, qsl], at2[mt][:, qsl], bc[:])

                o_proj_block(1)

    nc.compile()
    return nc


def get_program():
    global _PROG
    if _PROG is None:
        _PROG = _build_program()
    return _PROG


def _host_tables(bit_logits):
    """Replicate the reference fp32 cos/sin computation exactly (jax on CPU)."""
    import jax

    with jax.default_device(jax.devices("cpu")[0]):
        import jax.numpy as jnp

        basis = jnp.asarray(EULER_BASIS, dtype=jnp.float32)
        freqs = jax.nn.sigmoid(jnp.asarray(bit_logits, dtype=jnp.float32)) @ basis
        inv_freq = 2.0 ** (-(jnp.arange(0, DH, 2, dtype=jnp.float32) / DH))
        pos = jnp.arange(N, dtype=jnp.float32)
        theta = pos[None, :, None] * freqs[:, None, None] * inv_freq[None, None, :]
        cos = np.asarray(jnp.cos(theta))  # (H, N, 32)
        sin = np.asarray(jnp.sin(theta))
    return cos, sin


def _chunk_rows(a, p=128):
    """(R, C) -> (p, R//p, C); row r = kc*p + pp lands at [pp, kc]."""
    r, c = a.shape
    return np.ascontiguousarray(a.reshape(r // p, p, c).transpose(1, 0, 2))


def prepare_inputs(x, w_qkv, w_o, bit_logits):
    import ml_dtypes

    bf = ml_dtypes.bfloat16

    x = np.asarray(x, dtype=np.float32)
    w_qkv = np.asarray(w_qkv, dtype=np.float32)
    w_o = np.asarray(w_o, dtype=np.float32)
    cos, sin = _host_tables(np.asarray(bit_logits, dtype=np.float32))

    # de-interleave permutation within a head: evens then odds
    perm = np.concatenate([np.arange(0, DH, 2), np.arange(1, DH, 2)])

    wq_full = w_qkv.reshape(D, 3, H, DH)[:, 0]  # (D, H, DH)
    wk_full = w_qkv.reshape(D, 3, H, DH)[:, 1]
    wv_full = w_qkv.reshape(D, 3, H, DH)[:, 2]
    scale = 1.0 / math.sqrt(DH)

    # tri[krow, qcol] = 1 if qcol >= krow else 0, replicated 8x for the
    # strided diagonal mask
    tri = np.triu(np.ones((128, 128), dtype=np.float32))
    tri8 = np.broadcast_to(tri[:, None, :], (128, 8, 128)).copy()

    xT_by_batch = [
        _chunk_rows(np.ascontiguousarray(x[b].T)) for b in range(B)
    ]  # (128, KC, N)

    per_group = []
    for g in range(4):
        heads = range(4 * g, 4 * g + 4)
        wq_g = np.concatenate(
            [wq_full[:, h][:, perm] * scale for h in heads], axis=1
        )  # (D, 256)
        wk_g = np.concatenate([wk_full[:, h][:, perm] for h in heads], axis=1)
        wv_g = np.concatenate([wv_full[:, h] for h in heads], axis=1)
        wo_g = np.concatenate(
            [w_o.reshape(H, DH, D)[h] for h in heads], axis=0
        )  # (256, D)

        # rotation tables, layout (256 feats, N) -> (128, 2, N)
        ct = np.empty((DL, N), dtype=np.float32)
        st = np.empty((DL, N), dtype=np.float32)
        for hl, h in enumerate(heads):
            c = cos[h].T  # (32, N)
            s = sin[h].T
            ct[hl * DH : hl * DH + 32] = c
            ct[hl * DH + 32 : hl * DH + 64] = c
            st[hl * DH : hl * DH + 32] = -s
            st[hl * DH + 32 : hl * DH + 64] = s
        per_group.append(
            dict(
                wq=_chunk_rows(wq_g).astype(bf),
                wk=_chunk_rows(wk_g).astype(bf),
                wv=_chunk_rows(wv_g).astype(bf),
                wo=_chunk_rows(wo_g).astype(bf),
                ctab=_chunk_rows(ct).astype(bf),
                stab=_chunk_rows(st).astype(bf),
                tri8=tri8.astype(bf),
            )
        )

    in_maps = []
    for c in range(NCORES):
        b, g = c // 4, c % 4
        m = dict(per_group[g])
        m["xT"] = xT_by_batch[b].astype(bf)
        in_maps.append(m)
    return in_maps


def kernel(x, w_qkv, w_o, bit_logits, n_heads):
    global LAST_RESULTS
    from concourse.bass_utils import run_bass_kernel_spmd

    assert int(n_heads) == H
    nc = get_program()
    in_maps = prepare_inputs(x, w_qkv, w_o, bit_logits)
    res = run_bass_kernel_spmd(nc, in_maps, list(range(NCORES)))
    LAST_RESULTS = res
    out = np.zeros((B, N, D), dtype=np.float32)
    for c in range(NCORES):
        b = c // 4
        out[b] += res.results[c]["o_out"].reshape(N, D)
    return out


# revision 21
# speedup vs baseline: 1.0398x; 1.0398x over previous
"""EulerRotaryAttention Trainium2 kernel (bf16 matmul pipeline).

Sharding: 8 cores = 2 (batch) x 4 (head groups of 4 heads).  Each core
computes the qkv projection for its heads, rotary attention, and a partial
o-projection; the host sums partials over the 4 head groups per batch.

Device dataflow (zero on-device transposes):
  - x^T arrives pre-transposed from the host as (d, n), bf16.
  - Q^T, K^T computed directly in (feat, tok) layout with the projection
    weights as the stationary matmul operand; fp32 PSUM accumulation.
  - RoPE rotation applied during PSUM eviction.  Features are
    host-permuted (de-interleaved) so rotation pairs sit 32 partitions
    apart; cos/sin tables host-precomputed (replicating the reference
    fp32 arithmetic).  PSUM->bf16 cast on ScalarE, swap-half copies and
    multiply/add on VectorE in bf16 fast modes.
  - S^T in (k, q) layout (k on partitions, q free), causal tiles only;
    matmul streams are clipped to the causal column range per PSUM bank.
  - exp on ScalarE (scores ~ N(0,1): no max subtraction needed) into one
    (128, kt, 1024) bf16 tile per (head, q-chunk); the 8 diagonal 128x128
    subtiles are masked with a single strided tensor_tensor against a
    replicated 0/1 triangle.
  - PV: lhsT = [V | 1] (bf16) so the fp32 PSUM accumulator yields both
    A^T (feats on partitions, q free) and the softmax denominators.
  - denominators: batched VectorE reciprocal (no Ln -> only the Exp ACT
    table is ever loaded); GpSimd broadcast across partitions pairs two
    heads into one (128, 1024) scale tile; one in-place multiply
    normalizes each A^T head pair.
  - o-projection consumes A^T directly as lhsT; the partial (n, d) fp32
    output is written per core and summed on the host.
"""

import math

import numpy as np

B, N, D, H = 2, 2048, 1024, 16
DH = D // H  # 64
HL = 4  # local heads per core
DL = HL * DH  # 256 local features
KC = D // 128  # 8 contraction chunks
NT = N // 128  # 16 token tiles
NCH = N // 1024  # 2 wide column chunks
NCORES = 8

EULER_BASIS = (1.0, math.pi, math.e, math.pi * math.e, math.pi / math.e)

_PROG = None
LAST_RESULTS = None


def _build_program():
    import concourse.bass as bass
    import concourse.mybir as mybir
    import concourse.tile as tile
    from concourse import bacc

    f32 = mybir.dt.float32
    bf = mybir.dt.bfloat16
    AF = mybir.ActivationFunctionType

    nc = bacc.Bacc("TRN2", target_bir_lowering=False, num_devices=NCORES)

    xT = nc.declare_dram_parameter("xT", [128, KC, N], bf, isOutput=False)
    wq = nc.declare_dram_parameter("wq", [128, KC, DL], bf, isOutput=False)
    wk = nc.declare_dram_parameter("wk", [128, KC, DL], bf, isOutput=False)
    wv = nc.declare_dram_parameter("wv", [128, KC, DL], bf, isOutput=False)
    wo = nc.declare_dram_parameter("wo", [128, 2, D], bf, isOutput=False)
    ctab = nc.declare_dram_parameter("ctab", [128, 2, N], bf, isOutput=False)
    stab = nc.declare_dram_parameter("stab", [128, 2, N], bf, isOutput=False)
    tri8 = nc.declare_dram_parameter("tri8", [128, 8, 128], bf, isOutput=False)
    o_out = nc.declare_dram_parameter("o_out", [NT, 128, D], f32, isOutput=True)

    with tile.TileContext(nc) as tc:
        with tc.tile_pool(name="persist", bufs=1) as persist:
            # rotated Q^T / K^T: (256 feats, N) as 2 x (128, N), bf16
            qt_rot = [
                persist.tile([128, N], bf, tag=f"qt{m}", name=f"qt{m}")
                for m in range(2)
            ]
            kt_rot = [
                persist.tile([128, N], bf, tag=f"kt{m}", name=f"kt{m}")
                for m in range(2)
            ]
            # V for all heads with appended ones column: (128, NT, HL, 65)
            vones = persist.tile([128, NT, HL, DH + 1], bf, tag="vones", name="vones")
            nc.vector.memset(vones[:, :, :, DH : DH + 1], 1.0)
            # A^T head pairs: (128, N) bf16
            at2 = [
                persist.tile([128, N], bf, tag=f"at{m}", name=f"at{m}")
                for m in range(2)
            ]

            # ================= phase 1: projections =================
            with (
                tc.tile_pool(name="p1c", bufs=1) as p1c,
                tc.tile_pool(name="rot_tmp", bufs=3) as rot_tmp,
                tc.tile_pool(name="psum_qkt", bufs=2, space="PSUM") as psum_qkt,
                tc.tile_pool(name="psum_v", bufs=2, space="PSUM") as psum_v,
            ):
                wq_sb = p1c.tile([128, KC, DL], bf, tag="wq")
                wk_sb = p1c.tile([128, KC, DL], bf, tag="wk")
                wv_sb = p1c.tile([128, KC, DL], bf, tag="wv")
                ctab_sb = p1c.tile([128, 2, N], bf, tag="ctab")
                stab_sb = p1c.tile([128, 2, N], bf, tag="stab")
                xT_sb = p1c.tile([128, KC, N], bf, tag="xT")
                # order matters: the first QKT matmuls need wq + early xT
                # chunks; tables are only needed at the first eviction
                nc.sync.dma_start(out=wq_sb[:], in_=wq[:])
                for kc in range(KC):
                    nc.sync.dma_start(out=xT_sb[:, kc, :], in_=xT[:, kc, :])
                nc.sync.dma_start(out=wk_sb[:], in_=wk[:])
                nc.sync.dma_start(out=wv_sb[:], in_=wv[:])
                nc.sync.dma_start(out=ctab_sb[:], in_=ctab[:])
                nc.sync.dma_start(out=stab_sb[:], in_=stab[:])

                # Q^T / K^T: lhsT = w[kc, feats], rhs = xT[kc, toks]
                for w_sb, rot in ((wq_sb, qt_rot), (wk_sb, kt_rot)):
                    for mt in range(2):
                        for nh in range(NCH):  # 1024-wide tok chunks
                            nsl = slice(nh * 1024, (nh + 1) * 1024)
                            psum = psum_qkt.tile([128, 1024], f32, tag="qkt")
                            for kc in range(KC):
                                for nq in range(2):
                                    nc.tensor.matmul(
                                        psum[:, nq * 512 : (nq + 1) * 512],
                                        w_sb[:, kc, mt * 128 : (mt + 1) * 128],
                                        xT_sb[
                                            :,
                                            kc,
                                            nh * 1024
                                            + nq * 512 : nh * 1024
                                            + (nq + 1) * 512,
                                        ],
                                        start=(kc == 0),
                                        stop=(kc == KC - 1),
                                    )
                            # rotation eviction:
                            #   rot = raw * ctab + swap32(raw) * stab
                            raw = rot_tmp.tile([128, 1024], bf, tag="raw", name="raw")
                            nc.vector.tensor_copy(out=raw[:], in_=psum[:])
                            nc.vector.tensor_mul(
                                rot[mt][:, nsl], raw[:], ctab_sb[:, mt, nsl]
                            )
                            raws = rot_tmp.tile([128, 1024], bf, tag="rs", name="raws")
                            for g in range(4):
                                s = g ^ 1
                                nc.vector.tensor_copy(
                                    raws[g * 32 : (g + 1) * 32, :],
                                    raw[s * 32 : (s + 1) * 32, :],
                                )
                            tmp = rot_tmp.tile([128, 1024], bf, tag="rt", name="tmp")
                            nc.vector.tensor_mul(tmp[:], raws[:], stab_sb[:, mt, nsl])
                            nc.vector.tensor_add(
                                rot[mt][:, nsl], rot[mt][:, nsl], tmp[:]
                            )

                # V: lhsT = xT[kc, toks], rhs = wv[kc, feats]
                for tt in range(NT):
                    vpsum = psum_v.tile([128, DL], f32, tag="v")
                    for kc in range(KC):
                        nc.tensor.matmul(
                            vpsum[:],
                            xT_sb[:, kc, tt * 128 : (tt + 1) * 128],
                            wv_sb[:, kc, :],
                            start=(kc == 0),
                            stop=(kc == KC - 1),
                        )
                    # single strided eviction for all 4 heads of this tile
                    nc.scalar.copy(
                        out=vones[:, tt, :, 0:DH],
                        in_=vpsum[:].rearrange("p (h d) -> p h d", h=HL),
                    )

            # ============ phase 2a: attention ============
            with (
                tc.tile_pool(name="p2c", bufs=1) as p2c,
                tc.tile_pool(name="exps_pool", bufs=2) as exps_pool,
                tc.tile_pool(name="norm_pool", bufs=2) as norm_pool,
                tc.tile_pool(name="bcast_pool", bufs=2) as bcast_pool,
                tc.tile_pool(name="dscr_pool", bufs=4, space="DRAM") as dscr_pool,
                tc.tile_pool(name="ostage_pool", bufs=3) as ostage_pool,
                tc.tile_pool(name="psum_s", bufs=2, space="PSUM") as psum_s,
                tc.tile_pool(name="psum_pv", bufs=1, space="PSUM") as psum_pv,
                tc.tile_pool(name="psum_o", bufs=1, space="PSUM") as psum_o,
            ):
                tri8_sb = p2c.tile([128, 8, 128], bf, tag="tri8")
                wo_sb = p2c.tile([128, 2, D], bf, tag="wo")
                nc.sync.dma_start(out=tri8_sb[:], in_=tri8[:])
                nc.sync.dma_start(out=wo_sb[:], in_=wo[:])

                def o_proj_block(qc):
                    # o-projection for the 8 token tiles whose A^T columns
                    # were normalized by q-chunk qc
                    for tt in range(8 * qc, 8 * qc + 8):
                        opsum = psum_o.tile([128, D], f32, tag="o", name="opsum")
                        for hp in range(2):
                            for nb in range(2):
                                nc.tensor.matmul(
                                    opsum[:, nb * 512 : (nb + 1) * 512],
                                    at2[hp][:, tt * 128 : (tt + 1) * 128],
                                    wo_sb[:, hp, nb * 512 : (nb + 1) * 512],
                                    start=(hp == 0),
                                    stop=(hp == 1),
                                )
                        ost = ostage_pool.tile([128, D], f32, tag="ost", name="ost")
                        nc.scalar.copy(out=ost[:, 0:512], in_=opsum[:, 0:512])
                        nc.vector.tensor_copy(out=ost[:, 512:D], in_=opsum[:, 512:D])
                        nc.sync.dma_start(out=o_out[tt], in_=ost[:])

                for qch in range(NCH):
                    qsl = slice(qch * 1024, (qch + 1) * 1024)
                    nkt = 8 * qch + 8
                    # denominator rows live at partitions 0/32/64/96 (the
                    # only legal engine start partitions); unused rows are
                    # memset to 1.0 so the batched reciprocal stays finite
                    dnm4 = norm_pool.tile([97, 1024], f32, tag="dnm", name="dnm4")
                    rcp4 = norm_pool.tile([97, 1024], f32, tag="rcp", name="rcp4")
                    nc.gpsimd.memset(dnm4[:], 1.0)
                    for h in range(HL):
                        mt, roff = h // 2, (h % 2) * 64
                        if qch == 1 and h == 1:
                            # previous q-chunk's A^T is normalized by now;
                            # its o-projection overlaps this chunk's attention
                            o_proj_block(0)
                        exps = exps_pool.tile([128, NT, 1024], bf, tag="e", name="exps")
                        for kt in range(nkt):
                            j = kt - 8 * qch
                            jo = max(j, 0) * 128
                            spsum = psum_s.tile([128, 1024], f32, tag="s", name="spsum")
                            for nq in range(2):
                                lo = max(jo, nq * 512)
                                hi = (nq + 1) * 512
                                if lo >= hi:
                                    continue
                                nc.tensor.matmul(
                                    spsum[:, lo:hi],
                                    kt_rot[mt][
                                        roff : roff + 64, kt * 128 : (kt + 1) * 128
                                    ],
                                    qt_rot[mt][
                                        roff : roff + 64,
                                        qch * 1024 + lo : qch * 1024 + hi,
                                    ],
                                    start=True,
                                    stop=True,
                                )
                            nc.scalar.activation(
                                exps[:, kt, jo:1024], spsum[:, jo:1024], AF.Exp
                            )
                        # mask all 8 diagonal 128x128 subtiles in one op:
                        # element (p, j, c) -> exps[p, 8*qch + j, j*128 + c]
                        sub = exps[:, 8 * qch, :]
                        diag = bass.AP(
                            tensor=sub.tensor,
                            offset=sub.offset,
                            ap=[list(sub.ap[0]), [1152, 8], [1, 128]],
                        )
                        nc.vector.tensor_mul(diag, diag, tri8_sb[:])
                        # PV accumulation as one uninterrupted group
                        pv = psum_pv.tile([DH + 1, 1024], f32, tag="pv", name="pv")
                        # last kt contributing to each 512-col bank
                        last_kt = (8 * qch + 3, 8 * qch + 7)
                        for kt in range(nkt):
                            j = kt - 8 * qch
                            jo = max(j, 0) * 128
                            for nq in range(2):
                                lo = max(jo, nq * 512)
                                hi = (nq + 1) * 512
                                if lo >= hi:
                                    continue
                                nc.tensor.matmul(
                                    pv[:, lo:hi],
                                    vones[:, kt, h, :],
                                    exps[:, kt, lo:hi],
                                    start=(kt == 0),
                                    stop=(kt == last_kt[nq]),
                                )
                        # stash denominator, evict unnormalized A^T
                        nc.scalar.copy(
                            out=dnm4[32 * h : 32 * h + 1, :],
                            in_=pv[DH : DH + 1, :],
                        )
                        nc.vector.tensor_copy(
                            out=at2[mt][roff : roff + DH, qsl], in_=pv[0:DH, :]
                        )
                    # batched reciprocal of the 4 denominators
                    nc.vector.reciprocal(rcp4[:], dnm4[:])
                    for mt in range(2):
                        bc = bcast_pool.tile([128, 1024], f32, tag="bc", name="bc")
                        # broadcast each head's reciprocal row across 64
                        # partitions: bounce through DRAM, then a step-0
                        # partition DMA (legal for DRAM sources only; POOL's
                        # partition_broadcast ignores non-zero base
                        # partitions on hardware)
                        for half in range(2):
                            row = rcp4[64 * mt + 32 * half : 64 * mt + 32 * half + 1, :]
                            rdram = dscr_pool.tile([1, 1024], f32, tag="rd", name="rd")
                            nc.sync.dma_start(out=rdram[:], in_=row)
                            rd = rdram[:]
                            nc.sync.dma_start(
                                out=bc[64 * half : 64 * half + 64, :],
                                in_=bass.AP(
                                    tensor=rd.tensor,
                                    offset=rd.offset,
                                    ap=[[0, 64], [1, 1024]],
                                ),
                            )
                        nc.vector.tensor_mul(at2[mt][:, qsl], at2[mt][:, qsl], bc[:])

                o_proj_block(1)

    nc.compile()
    return nc


def get_program():
    global _PROG
    if _PROG is None:
        _PROG = _build_program()
    return _PROG


def _host_tables(bit_logits):
    """Replicate the reference fp32 cos/sin computation exactly (jax on CPU)."""
    import jax

    with jax.default_device(jax.devices("cpu")[0]):
        import jax.numpy as jnp

        basis = jnp.asarray(EULER_BASIS, dtype=jnp.float32)
        freqs = jax.nn.sigmoid(jnp.asarray(bit_logits, dtype=jnp.float32)) @ basis
        inv_freq = 2.0 ** (-(jnp.arange(0, DH, 2, dtype=jnp.float32) / DH))
        pos = jnp.arange(N, dtype=jnp.float32)
        theta = pos[None, :, None] * freqs[:, None, None] * inv_freq[None, None, :]
        cos = np.asarray(jnp.cos(theta))  # (H, N, 32)
        sin = np.asarray(jnp.sin(theta))
    return cos, sin


def _chunk_rows(a, p=128):
    """(R, C) -> (p, R//p, C); row r = kc*p + pp lands at [pp, kc]."""
    r, c = a.shape
    return np.ascontiguousarray(a.reshape(r // p, p, c).transpose(1, 0, 2))


def prepare_inputs(x, w_qkv, w_o, bit_logits):
    import ml_dtypes

    bf = ml_dtypes.bfloat16

    x = np.asarray(x, dtype=np.float32)
    w_qkv = np.asarray(w_qkv, dtype=np.float32)
    w_o = np.asarray(w_o, dtype=np.float32)
    cos, sin = _host_tables(np.asarray(bit_logits, dtype=np.float32))

    # de-interleave permutation within a head: evens then odds
    perm = np.concatenate([np.arange(0, DH, 2), np.arange(1, DH, 2)])

    wq_full = w_qkv.reshape(D, 3, H, DH)[:, 0]  # (D, H, DH)
    wk_full = w_qkv.reshape(D, 3, H, DH)[:, 1]
    wv_full = w_qkv.reshape(D, 3, H, DH)[:, 2]
    scale = 1.0 / math.sqrt(DH)

    # tri[krow, qcol] = 1 if qcol >= krow else 0, replicated 8x for the
    # strided diagonal mask
    tri = np.triu(np.ones((128, 128), dtype=np.float32))
    tri8 = np.broadcast_to(tri[:, None, :], (128, 8, 128)).copy()

    xT_by_batch = [
        _chunk_rows(np.ascontiguousarray(x[b].T)) for b in range(B)
    ]  # (128, KC, N)

    per_group = []
    for g in range(4):
        heads = range(4 * g, 4 * g + 4)
        wq_g = np.concatenate(
            [wq_full[:, h][:, perm] * scale for h in heads], axis=1
        )  # (D, 256)
        wk_g = np.concatenate([wk_full[:, h][:, perm] for h in heads], axis=1)
        wv_g = np.concatenate([wv_full[:, h] for h in heads], axis=1)
        wo_g = np.concatenate(
            [w_o.reshape(H, DH, D)[h] for h in heads], axis=0
        )  # (256, D)

        # rotation tables, layout (256 feats, N) -> (128, 2, N)
        ct = np.empty((DL, N), dtype=np.float32)
        st = np.empty((DL, N), dtype=np.float32)
        for hl, h in enumerate(heads):
            c = cos[h].T  # (32, N)
            s = sin[h].T
            ct[hl * DH : hl * DH + 32] = c
            ct[hl * DH + 32 : hl * DH + 64] = c
            st[hl * DH : hl * DH + 32] = -s
            st[hl * DH + 32 : hl * DH + 64] = s
        per_group.append(
            dict(
                wq=_chunk_rows(wq_g).astype(bf),
                wk=_chunk_rows(wk_g).astype(bf),
                wv=_chunk_rows(wv_g).astype(bf),
                wo=_chunk_rows(wo_g).astype(bf),
                ctab=_chunk_rows(ct).astype(bf),
                stab=_chunk_rows(st).astype(bf),
                tri8=tri8.astype(bf),
            )
        )

    in_maps = []
    for c in range(NCORES):
        b, g = c // 4, c % 4
        m = dict(per_group[g])
        m["xT"] = xT_by_batch[b].astype(bf)
        in_maps.append(m)
    return in_maps


def kernel(x, w_qkv, w_o, bit_logits, n_heads):
    global LAST_RESULTS
    from concourse.bass_utils import run_bass_kernel_spmd

    assert int(n_heads) == H
    nc = get_program()
    in_maps = prepare_inputs(x, w_qkv, w_o, bit_logits)
    res = run_bass_kernel_spmd(nc, in_maps, list(range(NCORES)))
    LAST_RESULTS = res
    out = np.zeros((B, N, D), dtype=np.float32)
    for c in range(NCORES):
        b = c // 4
        out[b] += res.results[c]["o_out"].reshape(N, D)
    return out


# revision 23
# speedup vs baseline: 1.1035x; 1.0612x over previous
"""EulerRotaryAttention Trainium2 kernel (bf16 matmul pipeline).

Sharding: 8 cores = 2 (batch) x 4 (head groups of 4 heads).  Each core
computes the qkv projection for its heads, rotary attention, and a partial
o-projection; the host sums partials over the 4 head groups per batch.

Device dataflow (zero on-device transposes):
  - x^T arrives pre-transposed from the host as (d, n), bf16.
  - Q^T, K^T computed directly in (feat, tok) layout with the projection
    weights as the stationary matmul operand; fp32 PSUM accumulation.
  - RoPE rotation applied during PSUM eviction.  Features are
    host-permuted (de-interleaved) so rotation pairs sit 32 partitions
    apart; cos/sin tables host-precomputed (replicating the reference
    fp32 arithmetic).  PSUM->bf16 cast on ScalarE, swap-half copies and
    multiply/add on VectorE in bf16 fast modes.
  - S^T in (k, q) layout (k on partitions, q free), causal tiles only;
    matmul streams are clipped to the causal column range per PSUM bank.
  - exp on ScalarE (scores ~ N(0,1): no max subtraction needed) into one
    (128, kt, 1024) bf16 tile per (head, q-chunk); the 8 diagonal 128x128
    subtiles are masked with a single strided tensor_tensor against a
    replicated 0/1 triangle.
  - PV: lhsT = [V | 1] (bf16) so the fp32 PSUM accumulator yields both
    A^T (feats on partitions, q free) and the softmax denominators.
  - denominators: batched VectorE reciprocal (no Ln -> only the Exp ACT
    table is ever loaded); GpSimd broadcast across partitions pairs two
    heads into one (128, 1024) scale tile; one in-place multiply
    normalizes each A^T head pair.
  - o-projection consumes A^T directly as lhsT; the partial (n, d) fp32
    output is written per core and summed on the host.
"""

import math

import numpy as np

B, N, D, H = 2, 2048, 1024, 16
DH = D // H  # 64
HL = 4  # local heads per core
DL = HL * DH  # 256 local features
KC = D // 128  # 8 contraction chunks
NT = N // 128  # 16 token tiles
NCH = N // 1024  # 2 wide column chunks
NCORES = 8

EULER_BASIS = (1.0, math.pi, math.e, math.pi * math.e, math.pi / math.e)

_PROG = None
LAST_RESULTS = None


def _build_program():
    import concourse.bass as bass
    import concourse.mybir as mybir
    import concourse.tile as tile
    from concourse import bacc

    f32 = mybir.dt.float32
    bf = mybir.dt.bfloat16
    AF = mybir.ActivationFunctionType

    nc = bacc.Bacc("TRN2", target_bir_lowering=False, num_devices=NCORES)

    xT = nc.declare_dram_parameter("xT", [128, KC, N], bf, isOutput=False)
    wq = nc.declare_dram_parameter("wq", [128, KC, DL], bf, isOutput=False)
    wk = nc.declare_dram_parameter("wk", [128, KC, DL], bf, isOutput=False)
    wv = nc.declare_dram_parameter("wv", [128, KC, DL], bf, isOutput=False)
    wo = nc.declare_dram_parameter("wo", [128, 2, D], bf, isOutput=False)
    ctab = nc.declare_dram_parameter("ctab", [128, 2, N], bf, isOutput=False)
    stab = nc.declare_dram_parameter("stab", [128, 2, N], bf, isOutput=False)
    tri8 = nc.declare_dram_parameter("tri8", [128, 8, 128], bf, isOutput=False)
    o_out = nc.declare_dram_parameter("o_out", [NT, 128, D], f32, isOutput=True)

    with tile.TileContext(nc) as tc:
        with tc.tile_pool(name="persist", bufs=1) as persist:
            # rotated Q^T / K^T: (256 feats, N) as 2 x (128, N), bf16
            qt_rot = [
                persist.tile([128, N], bf, tag=f"qt{m}", name=f"qt{m}")
                for m in range(2)
            ]
            kt_rot = [
                persist.tile([128, N], bf, tag=f"kt{m}", name=f"kt{m}")
                for m in range(2)
            ]
            # V for all heads with appended ones column: (128, NT, HL, 65)
            vones = persist.tile([128, NT, HL, DH + 1], bf, tag="vones", name="vones")
            nc.vector.memset(vones[:, :, :, DH : DH + 1], 1.0)
            # A^T head pairs: (128, N) bf16
            at2 = [
                persist.tile([128, N], bf, tag=f"at{m}", name=f"at{m}")
                for m in range(2)
            ]

            # ================= phase 1: projections =================
            with (
                tc.tile_pool(name="p1c", bufs=1) as p1c,
                tc.tile_pool(name="rot_tmp", bufs=3) as rot_tmp,
                tc.tile_pool(name="psum_qkt", bufs=2, space="PSUM") as psum_qkt,
                tc.tile_pool(name="psum_v", bufs=2, space="PSUM") as psum_v,
            ):
                wq_sb = p1c.tile([128, KC, DL], bf, tag="wq")
                wk_sb = p1c.tile([128, KC, DL], bf, tag="wk")
                wv_sb = p1c.tile([128, KC, DL], bf, tag="wv")
                ctab_sb = p1c.tile([128, 2, N], bf, tag="ctab")
                stab_sb = p1c.tile([128, 2, N], bf, tag="stab")
                xT_sb = p1c.tile([128, KC, N], bf, tag="xT")
                # order matters: the first QKT matmuls need wq + early xT
                # chunks; tables are only needed at the first eviction
                nc.sync.dma_start(out=wq_sb[:], in_=wq[:])
                for kc in range(KC):
                    nc.sync.dma_start(out=xT_sb[:, kc, :], in_=xT[:, kc, :])
                nc.sync.dma_start(out=wk_sb[:], in_=wk[:])
                nc.sync.dma_start(out=wv_sb[:], in_=wv[:])
                nc.sync.dma_start(out=ctab_sb[:], in_=ctab[:])
                nc.sync.dma_start(out=stab_sb[:], in_=stab[:])

                # Q^T / K^T: lhsT = w[kc, feats], rhs = xT[kc, toks]
                for w_sb, rot in ((wq_sb, qt_rot), (wk_sb, kt_rot)):
                    for mt in range(2):
                        for nh in range(NCH):  # 1024-wide tok chunks
                            nsl = slice(nh * 1024, (nh + 1) * 1024)
                            psum = psum_qkt.tile([128, 1024], f32, tag="qkt")
                            for kc in range(KC):
                                for nq in range(2):
                                    nc.tensor.matmul(
                                        psum[:, nq * 512 : (nq + 1) * 512],
                                        w_sb[:, kc, mt * 128 : (mt + 1) * 128],
                                        xT_sb[
                                            :,
                                            kc,
                                            nh * 1024
                                            + nq * 512 : nh * 1024
                                            + (nq + 1) * 512,
                                        ],
                                        start=(kc == 0),
                                        stop=(kc == KC - 1),
                                    )
                            # rotation eviction:
                            #   rot = raw * ctab + swap32(raw) * stab
                            raw = rot_tmp.tile([128, 1024], bf, tag="raw", name="raw")
                            nc.vector.tensor_copy(out=raw[:], in_=psum[:])
                            nc.vector.tensor_mul(
                                rot[mt][:, nsl], raw[:], ctab_sb[:, mt, nsl]
                            )
                            raws = rot_tmp.tile([128, 1024], bf, tag="rs", name="raws")
                            for g in range(4):
                                s = g ^ 1
                                nc.vector.tensor_copy(
                                    raws[g * 32 : (g + 1) * 32, :],
                                    raw[s * 32 : (s + 1) * 32, :],
                                )
                            tmp = rot_tmp.tile([128, 1024], bf, tag="rt", name="tmp")
                            nc.vector.tensor_mul(tmp[:], raws[:], stab_sb[:, mt, nsl])
                            nc.vector.tensor_add(
                                rot[mt][:, nsl], rot[mt][:, nsl], tmp[:]
                            )

                # V: lhsT = xT[kc, toks], rhs = wv[kc, feats]
                for tt in range(NT):
                    vpsum = psum_v.tile([128, DL], f32, tag="v")
                    for kc in range(KC):
                        nc.tensor.matmul(
                            vpsum[:],
                            xT_sb[:, kc, tt * 128 : (tt + 1) * 128],
                            wv_sb[:, kc, :],
                            start=(kc == 0),
                            stop=(kc == KC - 1),
                        )
                    # single strided eviction for all 4 heads of this tile
                    nc.scalar.copy(
                        out=vones[:, tt, :, 0:DH],
                        in_=vpsum[:].rearrange("p (h d) -> p h d", h=HL),
                    )

            # ============ phase 2a: attention ============
            with (
                tc.tile_pool(name="p2c", bufs=1) as p2c,
                tc.tile_pool(name="exps_pool", bufs=3) as exps_pool,
                tc.tile_pool(name="norm_pool", bufs=2) as norm_pool,
                tc.tile_pool(name="bcast_pool", bufs=2) as bcast_pool,
                tc.tile_pool(name="dscr_pool", bufs=4, space="DRAM") as dscr_pool,
                tc.tile_pool(name="ostage_pool", bufs=3) as ostage_pool,
                tc.tile_pool(name="psum_s", bufs=2, space="PSUM") as psum_s,
                tc.tile_pool(name="psum_pv", bufs=1, space="PSUM") as psum_pv,
                tc.tile_pool(name="psum_o", bufs=1, space="PSUM") as psum_o,
            ):
                tri8_sb = p2c.tile([128, 8, 128], bf, tag="tri8")
                wo_sb = p2c.tile([128, 2, D], bf, tag="wo")
                nc.sync.dma_start(out=tri8_sb[:], in_=tri8[:])
                nc.sync.dma_start(out=wo_sb[:], in_=wo[:])

                def o_proj_block(qc):
                    # o-projection for the 8 token tiles whose A^T columns
                    # were normalized by q-chunk qc
                    for tt in range(8 * qc, 8 * qc + 8):
                        opsum = psum_o.tile([128, D], f32, tag="o", name="opsum")
                        for hp in range(2):
                            for nb in range(2):
                                nc.tensor.matmul(
                                    opsum[:, nb * 512 : (nb + 1) * 512],
                                    at2[hp][:, tt * 128 : (tt + 1) * 128],
                                    wo_sb[:, hp, nb * 512 : (nb + 1) * 512],
                                    start=(hp == 0),
                                    stop=(hp == 1),
                                )
                        ost = ostage_pool.tile([128, D], f32, tag="ost", name="ost")
                        nc.scalar.copy(out=ost[:, 0:512], in_=opsum[:, 0:512])
                        nc.vector.tensor_copy(out=ost[:, 512:D], in_=opsum[:, 512:D])
                        nc.sync.dma_start(out=o_out[tt], in_=ost[:])

                dnm4s = {}

                def emit_s_exp(qch, h):
                    mt, roff = h // 2, (h % 2) * 64
                    nkt = 8 * qch + 8
                    exps = exps_pool.tile([128, NT, 1024], bf, tag="e", name="exps")
                    for kt in range(nkt):
                        j = kt - 8 * qch
                        jo = max(j, 0) * 128
                        spsum = psum_s.tile([128, 1024], f32, tag="s", name="spsum")
                        for nq in range(2):
                            lo = max(jo, nq * 512)
                            hi = (nq + 1) * 512
                            if lo >= hi:
                                continue
                            nc.tensor.matmul(
                                spsum[:, lo:hi],
                                kt_rot[mt][roff : roff + 64, kt * 128 : (kt + 1) * 128],
                                qt_rot[mt][
                                    roff : roff + 64,
                                    qch * 1024 + lo : qch * 1024 + hi,
                                ],
                                start=True,
                                stop=True,
                            )
                        nc.scalar.activation(
                            exps[:, kt, jo:1024], spsum[:, jo:1024], AF.Exp
                        )
                    # mask all 8 diagonal 128x128 subtiles in one op:
                    # element (p, j, c) -> exps[p, 8*qch + j, j*128 + c]
                    sub = exps[:, 8 * qch, :]
                    diag = bass.AP(
                        tensor=sub.tensor,
                        offset=sub.offset,
                        ap=[list(sub.ap[0]), [1152, 8], [1, 128]],
                    )
                    nc.vector.tensor_mul(diag, diag, tri8_sb[:])
                    return exps

                def emit_pv_evict(qch, h, exps):
                    mt, roff = h // 2, (h % 2) * 64
                    qsl = slice(qch * 1024, (qch + 1) * 1024)
                    nkt = 8 * qch + 8
                    if h == 0:
                        # denominator rows live at partitions 0/32/64/96
                        # (the only legal engine start partitions); unused
                        # rows are memset to 1.0 so the batched reciprocal
                        # stays finite
                        dnm4s[qch] = norm_pool.tile(
                            [97, 1024], f32, tag="dnm", name="dnm4"
                        )
                        nc.gpsimd.memset(dnm4s[qch][:], 1.0)
                    dnm4 = dnm4s[qch]
                    # PV accumulation as one uninterrupted group
                    pv = psum_pv.tile([DH + 1, 1024], f32, tag="pv", name="pv")
                    # last kt contributing to each 512-col bank
                    last_kt = (8 * qch + 3, 8 * qch + 7)
                    for kt in range(nkt):
                        j = kt - 8 * qch
                        jo = max(j, 0) * 128
                        for nq in range(2):
                            lo = max(jo, nq * 512)
                            hi = (nq + 1) * 512
                            if lo >= hi:
                                continue
                            nc.tensor.matmul(
                                pv[:, lo:hi],
                                vones[:, kt, h, :],
                                exps[:, kt, lo:hi],
                                start=(kt == 0),
                                stop=(kt == last_kt[nq]),
                            )
                    # stash denominator, evict unnormalized A^T
                    nc.scalar.copy(
                        out=dnm4[32 * h : 32 * h + 1, :], in_=pv[DH : DH + 1, :]
                    )
                    nc.vector.tensor_copy(
                        out=at2[mt][roff : roff + DH, qsl], in_=pv[0:DH, :]
                    )

                def emit_normalize(qch):
                    qsl = slice(qch * 1024, (qch + 1) * 1024)
                    rcp4 = norm_pool.tile([97, 1024], f32, tag="rcp", name="rcp4")
                    nc.vector.reciprocal(rcp4[:], dnm4s[qch][:])
                    for mt in range(2):
                        bc = bcast_pool.tile([128, 1024], f32, tag="bc", name="bc")
                        # broadcast each head's reciprocal row across 64
                        # partitions: bounce through DRAM, then a step-0
                        # partition DMA (legal for DRAM sources only; POOL's
                        # partition_broadcast ignores non-zero base
                        # partitions on hardware)
                        for half in range(2):
                            row = rcp4[64 * mt + 32 * half : 64 * mt + 32 * half + 1, :]
                            rdram = dscr_pool.tile([1, 1024], f32, tag="rd", name="rd")
                            nc.sync.dma_start(out=rdram[:], in_=row)
                            rd = rdram[:]
                            nc.sync.dma_start(
                                out=bc[64 * half : 64 * half + 64, :],
                                in_=bass.AP(
                                    tensor=rd.tensor,
                                    offset=rd.offset,
                                    ap=[[0, 64], [1, 1024]],
                                ),
                            )
                        nc.vector.tensor_mul(at2[mt][:, qsl], at2[mt][:, qsl], bc[:])

                # software-pipelined emission: S/exp of iteration i, then PV
                # of iteration i-1, so PV never waits on a fresh exp tail
                iters = [(qch, h) for qch in range(NCH) for h in range(HL)]
                pending = None  # (qch, h, exps)
                for qch, h in iters:
                    exps = emit_s_exp(qch, h)
                    if pending is not None:
                        emit_pv_evict(*pending)
                        if pending[1] == HL - 1:
                            emit_normalize(pending[0])
                            o_proj_block(pending[0])
                    pending = (qch, h, exps)
                emit_pv_evict(*pending)
                emit_normalize(pending[0])
                o_proj_block(pending[0])

    nc.compile()
    return nc


def get_program():
    global _PROG
    if _PROG is None:
        _PROG = _build_program()
    return _PROG


def _host_tables(bit_logits):
    """Replicate the reference fp32 cos/sin computation exactly (jax on CPU)."""
    import jax

    with jax.default_device(jax.devices("cpu")[0]):
        import jax.numpy as jnp

        basis = jnp.asarray(EULER_BASIS, dtype=jnp.float32)
        freqs = jax.nn.sigmoid(jnp.asarray(bit_logits, dtype=jnp.float32)) @ basis
        inv_freq = 2.0 ** (-(jnp.arange(0, DH, 2, dtype=jnp.float32) / DH))
        pos = jnp.arange(N, dtype=jnp.float32)
        theta = pos[None, :, None] * freqs[:, None, None] * inv_freq[None, None, :]
        cos = np.asarray(jnp.cos(theta))  # (H, N, 32)
        sin = np.asarray(jnp.sin(theta))
    return cos, sin


def _chunk_rows(a, p=128):
    """(R, C) -> (p, R//p, C); row r = kc*p + pp lands at [pp, kc]."""
    r, c = a.shape
    return np.ascontiguousarray(a.reshape(r // p, p, c).transpose(1, 0, 2))


def prepare_inputs(x, w_qkv, w_o, bit_logits):
    import ml_dtypes

    bf = ml_dtypes.bfloat16

    x = np.asarray(x, dtype=np.float32)
    w_qkv = np.asarray(w_qkv, dtype=np.float32)
    w_o = np.asarray(w_o, dtype=np.float32)
    cos, sin = _host_tables(np.asarray(bit_logits, dtype=np.float32))

    # de-interleave permutation within a head: evens then odds
    perm = np.concatenate([np.arange(0, DH, 2), np.arange(1, DH, 2)])

    wq_full = w_qkv.reshape(D, 3, H, DH)[:, 0]  # (D, H, DH)
    wk_full = w_qkv.reshape(D, 3, H, DH)[:, 1]
    wv_full = w_qkv.reshape(D, 3, H, DH)[:, 2]
    scale = 1.0 / math.sqrt(DH)

    # tri[krow, qcol] = 1 if qcol >= krow else 0, replicated 8x for the
    # strided diagonal mask
    tri = np.triu(np.ones((128, 128), dtype=np.float32))
    tri8 = np.broadcast_to(tri[:, None, :], (128, 8, 128)).copy()

    xT_by_batch = [
        _chunk_rows(np.ascontiguousarray(x[b].T)) for b in range(B)
    ]  # (128, KC, N)

    per_group = []
    for g in range(4):
        heads = range(4 * g, 4 * g + 4)
        wq_g = np.concatenate(
            [wq_full[:, h][:, perm] * scale for h in heads], axis=1
        )  # (D, 256)
        wk_g = np.concatenate([wk_full[:, h][:, perm] for h in heads], axis=1)
        wv_g = np.concatenate([wv_full[:, h] for h in heads], axis=1)
        wo_g = np.concatenate(
            [w_o.reshape(H, DH, D)[h] for h in heads], axis=0
        )  # (256, D)

        # rotation tables, layout (256 feats, N) -> (128, 2, N)
        ct = np.empty((DL, N), dtype=np.float32)
        st = np.empty((DL, N), dtype=np.float32)
        for hl, h in enumerate(heads):
            c = cos[h].T  # (32, N)
            s = sin[h].T
            ct[hl * DH : hl * DH + 32] = c
            ct[hl * DH + 32 : hl * DH + 64] = c
            st[hl * DH : hl * DH + 32] = -s
            st[hl * DH + 32 : hl * DH + 64] = s
        per_group.append(
            dict(
                wq=_chunk_rows(wq_g).astype(bf),
                wk=_chunk_rows(wk_g).astype(bf),
                wv=_chunk_rows(wv_g).astype(bf),
                wo=_chunk_rows(wo_g).astype(bf),
                ctab=_chunk_rows(ct).astype(bf),
                stab=_chunk_rows(st).astype(bf),
                tri8=tri8.astype(bf),
            )
        )

    in_maps = []
    for c in range(NCORES):
        b, g = c // 4, c % 4
        m = dict(per_group[g])
        m["xT"] = xT_by_batch[b].astype(bf)
        in_maps.append(m)
    return in_maps


def kernel(x, w_qkv, w_o, bit_logits, n_heads):
    global LAST_RESULTS
    from concourse.bass_utils import run_bass_kernel_spmd

    assert int(n_heads) == H
    nc = get_program()
    in_maps = prepare_inputs(x, w_qkv, w_o, bit_logits)
    res = run_bass_kernel_spmd(nc, in_maps, list(range(NCORES)))
    LAST_RESULTS = res
    out = np.zeros((B, N, D), dtype=np.float32)
    for c in range(NCORES):
        b = c // 4
        out[b] += res.results[c]["o_out"].reshape(N, D)
    return out


# revision 24
# speedup vs baseline: 1.1694x; 1.0597x over previous
"""EulerRotaryAttention Trainium2 kernel (bf16 matmul pipeline).

Sharding: 8 cores = 2 (batch) x 4 (head groups of 4 heads).  Each core
computes the qkv projection for its heads, rotary attention, and a partial
o-projection; the host sums partials over the 4 head groups per batch.

Device dataflow (zero on-device transposes):
  - x^T arrives pre-transposed from the host as (d, n), bf16.
  - Q^T, K^T computed directly in (feat, tok) layout with the projection
    weights as the stationary matmul operand; fp32 PSUM accumulation.
  - RoPE rotation applied during PSUM eviction.  Features are
    host-permuted (de-interleaved) so rotation pairs sit 32 partitions
    apart; cos/sin tables host-precomputed (replicating the reference
    fp32 arithmetic).  PSUM->bf16 cast on ScalarE, swap-half copies and
    multiply/add on VectorE in bf16 fast modes.
  - S^T in (k, q) layout (k on partitions, q free), causal tiles only;
    matmul streams are clipped to the causal column range per PSUM bank.
  - exp on ScalarE (scores ~ N(0,1): no max subtraction needed) into one
    (128, kt, 1024) bf16 tile per (head, q-chunk); the 8 diagonal 128x128
    subtiles are masked with a single strided tensor_tensor against a
    replicated 0/1 triangle.
  - PV: lhsT = [V | 1] (bf16) so the fp32 PSUM accumulator yields both
    A^T (feats on partitions, q free) and the softmax denominators.
  - denominators: batched VectorE reciprocal (no Ln -> only the Exp ACT
    table is ever loaded); GpSimd broadcast across partitions pairs two
    heads into one (128, 1024) scale tile; one in-place multiply
    normalizes each A^T head pair.
  - o-projection consumes A^T directly as lhsT; the partial (n, d) fp32
    output is written per core and summed on the host.
"""

import math

import numpy as np

B, N, D, H = 2, 2048, 1024, 16
DH = D // H  # 64
HL = 4  # local heads per core
DL = HL * DH  # 256 local features
KC = D // 128  # 8 contraction chunks
NT = N // 128  # 16 token tiles
NCH = N // 1024  # 2 wide column chunks
NCORES = 8

EULER_BASIS = (1.0, math.pi, math.e, math.pi * math.e, math.pi / math.e)

_PROG = None
LAST_RESULTS = None


def _build_program():
    import concourse.bass as bass
    import concourse.mybir as mybir
    import concourse.tile as tile
    from concourse import bacc

    f32 = mybir.dt.float32
    bf = mybir.dt.bfloat16
    AF = mybir.ActivationFunctionType

    nc = bacc.Bacc("TRN2", target_bir_lowering=False, num_devices=NCORES)

    xT = nc.declare_dram_parameter("xT", [128, KC, N], bf, isOutput=False)
    wq = nc.declare_dram_parameter("wq", [128, KC, DL], bf, isOutput=False)
    wk = nc.declare_dram_parameter("wk", [128, KC, DL], bf, isOutput=False)
    wv = nc.declare_dram_parameter("wv", [128, KC, DL], bf, isOutput=False)
    wo = nc.declare_dram_parameter("wo", [128, 2, D], bf, isOutput=False)
    ctab = nc.declare_dram_parameter("ctab", [128, 2, N], bf, isOutput=False)
    stab = nc.declare_dram_parameter("stab", [128, 2, N], bf, isOutput=False)
    tri8 = nc.declare_dram_parameter("tri8", [128, 8, 128], bf, isOutput=False)
    o_out = nc.declare_dram_parameter("o_out", [NT, 128, D], bf, isOutput=True)

    with tile.TileContext(nc) as tc:
        with tc.tile_pool(name="persist", bufs=1) as persist:
            # rotated Q^T / K^T: (256 feats, N) as 2 x (128, N), bf16
            qt_rot = [
                persist.tile([128, N], bf, tag=f"qt{m}", name=f"qt{m}")
                for m in range(2)
            ]
            kt_rot = [
                persist.tile([128, N], bf, tag=f"kt{m}", name=f"kt{m}")
                for m in range(2)
            ]
            # V for all heads with appended ones column: (128, NT, HL, 65)
            vones = persist.tile([128, NT, HL, DH + 1], bf, tag="vones", name="vones")
            nc.vector.memset(vones[:, :, :, DH : DH + 1], 1.0)
            # A^T head pairs: (128, N) bf16
            at2 = [
                persist.tile([128, N], bf, tag=f"at{m}", name=f"at{m}")
                for m in range(2)
            ]

            # ================= phase 1: projections =================
            with (
                tc.tile_pool(name="p1c", bufs=1) as p1c,
                tc.tile_pool(name="rot_tmp", bufs=3) as rot_tmp,
                tc.tile_pool(name="psum_qkt", bufs=2, space="PSUM") as psum_qkt,
                tc.tile_pool(name="psum_v", bufs=2, space="PSUM") as psum_v,
            ):
                wq_sb = p1c.tile([128, KC, DL], bf, tag="wq")
                wk_sb = p1c.tile([128, KC, DL], bf, tag="wk")
                wv_sb = p1c.tile([128, KC, DL], bf, tag="wv")
                ctab_sb = p1c.tile([128, 2, N], bf, tag="ctab")
                stab_sb = p1c.tile([128, 2, N], bf, tag="stab")
                xT_sb = p1c.tile([128, KC, N], bf, tag="xT")
                # order matters: the first QKT matmuls need wq + early xT
                # chunks; tables are only needed at the first eviction
                nc.sync.dma_start(out=wq_sb[:], in_=wq[:])
                for kc in range(KC):
                    nc.sync.dma_start(out=xT_sb[:, kc, :], in_=xT[:, kc, :])
                nc.sync.dma_start(out=wk_sb[:], in_=wk[:])
                nc.sync.dma_start(out=wv_sb[:], in_=wv[:])
                nc.sync.dma_start(out=ctab_sb[:], in_=ctab[:])
                nc.sync.dma_start(out=stab_sb[:], in_=stab[:])

                # Q^T / K^T: lhsT = w[kc, feats], rhs = xT[kc, toks]
                for w_sb, rot in ((wq_sb, qt_rot), (wk_sb, kt_rot)):
                    for mt in range(2):
                        for nh in range(NCH):  # 1024-wide tok chunks
                            nsl = slice(nh * 1024, (nh + 1) * 1024)
                            psum = psum_qkt.tile([128, 1024], f32, tag="qkt")
                            for kc in range(KC):
                                for nq in range(2):
                                    nc.tensor.matmul(
                                        psum[:, nq * 512 : (nq + 1) * 512],
                                        w_sb[:, kc, mt * 128 : (mt + 1) * 128],
                                        xT_sb[
                                            :,
                                            kc,
                                            nh * 1024
                                            + nq * 512 : nh * 1024
                                            + (nq + 1) * 512,
                                        ],
                                        start=(kc == 0),
                                        stop=(kc == KC - 1),
                                    )
                            # rotation eviction:
                            #   rot = raw * ctab + swap32(raw) * stab
                            raw = rot_tmp.tile([128, 1024], bf, tag="raw", name="raw")
                            nc.scalar.copy(out=raw[:], in_=psum[:])
                            nc.vector.tensor_mul(
                                rot[mt][:, nsl], raw[:], ctab_sb[:, mt, nsl]
                            )
                            raws = rot_tmp.tile([128, 1024], bf, tag="rs", name="raws")
                            for g in range(4):
                                s = g ^ 1
                                nc.vector.tensor_copy(
                                    raws[g * 32 : (g + 1) * 32, :],
                                    raw[s * 32 : (s + 1) * 32, :],
                                )
                            tmp = rot_tmp.tile([128, 1024], bf, tag="rt", name="tmp")
                            nc.vector.tensor_mul(tmp[:], raws[:], stab_sb[:, mt, nsl])
                            nc.vector.tensor_add(
                                rot[mt][:, nsl], rot[mt][:, nsl], tmp[:]
                            )

                # V: lhsT = xT[kc, toks], rhs = wv[kc, feats]
                for tt in range(NT):
                    vpsum = psum_v.tile([128, DL], f32, tag="v")
                    for kc in range(KC):
                        nc.tensor.matmul(
                            vpsum[:],
                            xT_sb[:, kc, tt * 128 : (tt + 1) * 128],
                            wv_sb[:, kc, :],
                            start=(kc == 0),
                            stop=(kc == KC - 1),
                        )
                    # single strided eviction for all 4 heads of this tile
                    nc.scalar.copy(
                        out=vones[:, tt, :, 0:DH],
                        in_=vpsum[:].rearrange("p (h d) -> p h d", h=HL),
                    )

            # ============ phase 2a: attention ============
            with (
                tc.tile_pool(name="p2c", bufs=1) as p2c,
                tc.tile_pool(name="exps_pool", bufs=3) as exps_pool,
                tc.tile_pool(name="norm_pool", bufs=2) as norm_pool,
                tc.tile_pool(name="bcast_pool", bufs=2) as bcast_pool,
                tc.tile_pool(name="dscr_pool", bufs=4, space="DRAM") as dscr_pool,
                tc.tile_pool(name="ostage_pool", bufs=3) as ostage_pool,
                tc.tile_pool(name="psum_s", bufs=2, space="PSUM") as psum_s,
                tc.tile_pool(name="psum_pv", bufs=1, space="PSUM") as psum_pv,
                tc.tile_pool(name="psum_o", bufs=2, space="PSUM") as psum_o,
            ):
                tri8_sb = p2c.tile([128, 8, 128], bf, tag="tri8")
                wo_sb = p2c.tile([128, 2, D], bf, tag="wo")
                nc.sync.dma_start(out=tri8_sb[:], in_=tri8[:])
                nc.sync.dma_start(out=wo_sb[:], in_=wo[:])

                def o_proj_block(qc):
                    # o-projection for the 8 token tiles whose A^T columns
                    # were normalized by q-chunk qc
                    for tt in range(8 * qc, 8 * qc + 8):
                        ost = ostage_pool.tile([128, D], bf, tag="ost", name="ost")
                        for nb in range(2):
                            opsum = psum_o.tile([128, 512], f32, tag="o", name="opsum")
                            for hp in range(2):
                                nc.tensor.matmul(
                                    opsum[:],
                                    at2[hp][:, tt * 128 : (tt + 1) * 128],
                                    wo_sb[:, hp, nb * 512 : (nb + 1) * 512],
                                    start=(hp == 0),
                                    stop=(hp == 1),
                                )
                            if nb == 0:
                                nc.scalar.copy(
                                    out=ost[:, 0:512], in_=opsum[:]
                                )
                            else:
                                nc.vector.tensor_copy(
                                    out=ost[:, 512:D], in_=opsum[:]
                                )
                        nc.sync.dma_start(out=o_out[tt], in_=ost[:])

                dnm4s = {}

                def emit_s_exp(qch, h):
                    mt, roff = h // 2, (h % 2) * 64
                    nkt = 8 * qch + 8
                    exps = exps_pool.tile([128, NT, 1024], bf, tag="e", name="exps")
                    for kt in range(nkt):
                        j = kt - 8 * qch
                        jo = max(j, 0) * 128
                        spsum = psum_s.tile([128, 1024], f32, tag="s", name="spsum")
                        for nq in range(2):
                            lo = max(jo, nq * 512)
                            hi = (nq + 1) * 512
                            if lo >= hi:
                                continue
                            nc.tensor.matmul(
                                spsum[:, lo:hi],
                                kt_rot[mt][roff : roff + 64, kt * 128 : (kt + 1) * 128],
                                qt_rot[mt][
                                    roff : roff + 64,
                                    qch * 1024 + lo : qch * 1024 + hi,
                                ],
                                start=True,
                                stop=True,
                            )
                        nc.scalar.activation(
                            exps[:, kt, jo:1024], spsum[:, jo:1024], AF.Exp
                        )
                    # mask all 8 diagonal 128x128 subtiles in one op:
                    # element (p, j, c) -> exps[p, 8*qch + j, j*128 + c]
                    sub = exps[:, 8 * qch, :]
                    diag = bass.AP(
                        tensor=sub.tensor,
                        offset=sub.offset,
                        ap=[list(sub.ap[0]), [1152, 8], [1, 128]],
                    )
                    nc.vector.tensor_mul(diag, diag, tri8_sb[:])
                    return exps

                def emit_pv_evict(qch, h, exps):
                    mt, roff = h // 2, (h % 2) * 64
                    qsl = slice(qch * 1024, (qch + 1) * 1024)
                    nkt = 8 * qch + 8
                    if h == 0:
                        # denominator rows live at partitions 0/32/64/96
                        # (the only legal engine start partitions); unused
                        # rows are memset to 1.0 so the batched reciprocal
                        # stays finite
                        dnm4s[qch] = norm_pool.tile(
                            [97, 1024], f32, tag="dnm", name="dnm4"
                        )
                        nc.gpsimd.memset(dnm4s[qch][:], 1.0)
                    dnm4 = dnm4s[qch]
                    # PV accumulation as one uninterrupted group
                    pv = psum_pv.tile([DH + 1, 1024], f32, tag="pv", name="pv")
                    # last kt contributing to each 512-col bank
                    last_kt = (8 * qch + 3, 8 * qch + 7)
                    for kt in range(nkt):
                        j = kt - 8 * qch
                        jo = max(j, 0) * 128
                        for nq in range(2):
                            lo = max(jo, nq * 512)
                            hi = (nq + 1) * 512
                            if lo >= hi:
                                continue
                            nc.tensor.matmul(
                                pv[:, lo:hi],
                                vones[:, kt, h, :],
                                exps[:, kt, lo:hi],
                                start=(kt == 0),
                                stop=(kt == last_kt[nq]),
                            )
                    # stash denominator, evict unnormalized A^T
                    nc.scalar.copy(
                        out=dnm4[32 * h : 32 * h + 1, :], in_=pv[DH : DH + 1, :]
                    )
                    nc.vector.tensor_copy(
                        out=at2[mt][roff : roff + DH, qsl], in_=pv[0:DH, :]
                    )

                def emit_normalize(qch):
                    qsl = slice(qch * 1024, (qch + 1) * 1024)
                    rcp4 = norm_pool.tile([97, 1024], f32, tag="rcp", name="rcp4")
                    nc.vector.reciprocal(rcp4[:], dnm4s[qch][:])
                    for mt in range(2):
                        bc = bcast_pool.tile([128, 1024], f32, tag="bc", name="bc")
                        # broadcast each head's reciprocal row across 64
                        # partitions: bounce through DRAM, then a step-0
                        # partition DMA (legal for DRAM sources only; POOL's
                        # partition_broadcast ignores non-zero base
                        # partitions on hardware)
                        for half in range(2):
                            row = rcp4[64 * mt + 32 * half : 64 * mt + 32 * half + 1, :]
                            rdram = dscr_pool.tile([1, 1024], f32, tag="rd", name="rd")
                            nc.sync.dma_start(out=rdram[:], in_=row)
                            rd = rdram[:]
                            nc.sync.dma_start(
                                out=bc[64 * half : 64 * half + 64, :],
                                in_=bass.AP(
                                    tensor=rd.tensor,
                                    offset=rd.offset,
                                    ap=[[0, 64], [1, 1024]],
                                ),
                            )
                        nc.vector.tensor_mul(at2[mt][:, qsl], at2[mt][:, qsl], bc[:])

                # software-pipelined emission: S/exp of iteration i, then PV
                # of iteration i-1, so PV never waits on a fresh exp tail
                iters = [(qch, h) for qch in range(NCH) for h in range(HL)]
                pending = None  # (qch, h, exps)
                for qch, h in iters:
                    exps = emit_s_exp(qch, h)
                    if pending is not None:
                        emit_pv_evict(*pending)
                        if pending[1] == HL - 1:
                            emit_normalize(pending[0])
                            o_proj_block(pending[0])
                    pending = (qch, h, exps)
                emit_pv_evict(*pending)
                emit_normalize(pending[0])
                o_proj_block(pending[0])

    nc.compile()
    return nc


def get_program():
    global _PROG
    if _PROG is None:
        _PROG = _build_program()
    return _PROG


def _host_tables(bit_logits):
    """Replicate the reference fp32 cos/sin computation exactly (jax on CPU)."""
    import jax

    with jax.default_device(jax.devices("cpu")[0]):
        import jax.numpy as jnp

        basis = jnp.asarray(EULER_BASIS, dtype=jnp.float32)
        freqs = jax.nn.sigmoid(jnp.asarray(bit_logits, dtype=jnp.float32)) @ basis
        inv_freq = 2.0 ** (-(jnp.arange(0, DH, 2, dtype=jnp.float32) / DH))
        pos = jnp.arange(N, dtype=jnp.float32)
        theta = pos[None, :, None] * freqs[:, None, None] * inv_freq[None, None, :]
        cos = np.asarray(jnp.cos(theta))  # (H, N, 32)
        sin = np.asarray(jnp.sin(theta))
    return cos, sin


def _chunk_rows(a, p=128):
    """(R, C) -> (p, R//p, C); row r = kc*p + pp lands at [pp, kc]."""
    r, c = a.shape
    return np.ascontiguousarray(a.reshape(r // p, p, c).transpose(1, 0, 2))


def prepare_inputs(x, w_qkv, w_o, bit_logits):
    import ml_dtypes

    bf = ml_dtypes.bfloat16

    x = np.asarray(x, dtype=np.float32)
    w_qkv = np.asarray(w_qkv, dtype=np.float32)
    w_o = np.asarray(w_o, dtype=np.float32)
    cos, sin = _host_tables(np.asarray(bit_logits, dtype=np.float32))

    # de-interleave permutation within a head: evens then odds
    perm = np.concatenate([np.arange(0, DH, 2), np.arange(1, DH, 2)])

    wq_full = w_qkv.reshape(D, 3, H, DH)[:, 0]  # (D, H, DH)
    wk_full = w_qkv.reshape(D, 3, H, DH)[:, 1]
    wv_full = w_qkv.reshape(D, 3, H, DH)[:, 2]
    scale = 1.0 / math.sqrt(DH)

    # tri[krow, qcol] = 1 if qcol >= krow else 0, replicated 8x for the
    # strided diagonal mask
    tri = np.triu(np.ones((128, 128), dtype=np.float32))
    tri8 = np.broadcast_to(tri[:, None, :], (128, 8, 128)).copy()

    xT_by_batch = [
        _chunk_rows(np.ascontiguousarray(x[b].T)) for b in range(B)
    ]  # (128, KC, N)

    per_group = []
    for g in range(4):
        heads = range(4 * g, 4 * g + 4)
        wq_g = np.concatenate(
            [wq_full[:, h][:, perm] * scale for h in heads], axis=1
        )  # (D, 256)
        wk_g = np.concatenate([wk_full[:, h][:, perm] for h in heads], axis=1)
        wv_g = np.concatenate([wv_full[:, h] for h in heads], axis=1)
        wo_g = np.concatenate(
            [w_o.reshape(H, DH, D)[h] for h in heads], axis=0
        )  # (256, D)

        # rotation tables, layout (256 feats, N) -> (128, 2, N)
        ct = np.empty((DL, N), dtype=np.float32)
        st = np.empty((DL, N), dtype=np.float32)
        for hl, h in enumerate(heads):
            c = cos[h].T  # (32, N)
            s = sin[h].T
            ct[hl * DH : hl * DH + 32] = c
            ct[hl * DH + 32 : hl * DH + 64] = c
            st[hl * DH : hl * DH + 32] = -s
            st[hl * DH + 32 : hl * DH + 64] = s
        per_group.append(
            dict(
                wq=_chunk_rows(wq_g).astype(bf),
                wk=_chunk_rows(wk_g).astype(bf),
                wv=_chunk_rows(wv_g).astype(bf),
                wo=_chunk_rows(wo_g).astype(bf),
                ctab=_chunk_rows(ct).astype(bf),
                stab=_chunk_rows(st).astype(bf),
                tri8=tri8.astype(bf),
            )
        )

    in_maps = []
    for c in range(NCORES):
        b, g = c // 4, c % 4
        m = dict(per_group[g])
        m["xT"] = xT_by_batch[b].astype(bf)
        in_maps.append(m)
    return in_maps


def kernel(x, w_qkv, w_o, bit_logits, n_heads):
    global LAST_RESULTS
    from concourse.bass_utils import run_bass_kernel_spmd

    assert int(n_heads) == H
    nc = get_program()
    in_maps = prepare_inputs(x, w_qkv, w_o, bit_logits)
    res = run_bass_kernel_spmd(nc, in_maps, list(range(NCORES)))
    LAST_RESULTS = res
    out = np.zeros((B, N, D), dtype=np.float32)
    for c in range(NCORES):
        b = c // 4
        out[b] += res.results[c]["o_out"].reshape(N, D).astype(np.float32)
    return out


# revision 27
# speedup vs baseline: 1.1927x; 1.0199x over previous
"""EulerRotaryAttention Trainium2 kernel (bf16 matmul pipeline).

Sharding: 8 cores = 2 (batch) x 4 (head groups of 4 heads).  Each core
computes the qkv projection for its heads, rotary attention, and a partial
o-projection; the host sums partials over the 4 head groups per batch.

Device dataflow (zero on-device transposes):
  - x^T arrives pre-transposed from the host as (d, n), bf16.
  - Q^T, K^T computed directly in (feat, tok) layout with the projection
    weights as the stationary matmul operand; fp32 PSUM accumulation.
  - RoPE rotation applied during PSUM eviction.  Features are
    host-permuted (de-interleaved) so rotation pairs sit 32 partitions
    apart; cos/sin tables host-precomputed (replicating the reference
    fp32 arithmetic).  PSUM->bf16 cast on ScalarE, swap-half copies and
    multiply/add on VectorE in bf16 fast modes.
  - S^T in (k, q) layout (k on partitions, q free), causal tiles only;
    matmul streams are clipped to the causal column range per PSUM bank.
  - exp on ScalarE (scores ~ N(0,1): no max subtraction needed) into one
    (128, kt, 1024) bf16 tile per (head, q-chunk); the 8 diagonal 128x128
    subtiles are masked with a single strided tensor_tensor against a
    replicated 0/1 triangle.
  - PV: lhsT = [V | 1] (bf16) so the fp32 PSUM accumulator yields both
    A^T (feats on partitions, q free) and the softmax denominators.
  - denominators: batched VectorE reciprocal (no Ln -> only the Exp ACT
    table is ever loaded); GpSimd broadcast across partitions pairs two
    heads into one (128, 1024) scale tile; one in-place multiply
    normalizes each A^T head pair.
  - o-projection consumes A^T directly as lhsT; the partial (n, d) fp32
    output is written per core and summed on the host.
"""

import math

import numpy as np

B, N, D, H = 2, 2048, 1024, 16
DH = D // H  # 64
HL = 4  # local heads per core
DL = HL * DH  # 256 local features
KC = D // 128  # 8 contraction chunks
NT = N // 128  # 16 token tiles
NCH = N // 1024  # 2 wide column chunks
NCORES = 8

EULER_BASIS = (1.0, math.pi, math.e, math.pi * math.e, math.pi / math.e)

_PROG = None
LAST_RESULTS = None


def _build_program():
    import concourse.bass as bass
    import concourse.mybir as mybir
    import concourse.tile as tile
    from concourse import bacc

    f32 = mybir.dt.float32
    bf = mybir.dt.bfloat16
    AF = mybir.ActivationFunctionType

    nc = bacc.Bacc("TRN2", target_bir_lowering=False, num_devices=NCORES)

    xT = nc.declare_dram_parameter("xT", [128, KC, N], bf, isOutput=False)
    wq = nc.declare_dram_parameter("wq", [128, KC, DL], bf, isOutput=False)
    wk = nc.declare_dram_parameter("wk", [128, KC, DL], bf, isOutput=False)
    wv = nc.declare_dram_parameter("wv", [128, KC, DL], bf, isOutput=False)
    wo = nc.declare_dram_parameter("wo", [128, 2, D], bf, isOutput=False)
    ctab = nc.declare_dram_parameter("ctab", [128, 2, N], bf, isOutput=False)
    stab = nc.declare_dram_parameter("stab", [128, 2, N], bf, isOutput=False)
    tri8 = nc.declare_dram_parameter("tri8", [128, 8, 128], bf, isOutput=False)
    o_out = nc.declare_dram_parameter("o_out", [NT, 128, D], bf, isOutput=True)

    with tile.TileContext(nc) as tc:
        with tc.tile_pool(name="persist", bufs=1) as persist:
            # rotated Q^T / K^T: (256 feats, N) as 2 x (128, N), bf16
            qt_rot = [
                persist.tile([128, N], bf, tag=f"qt{m}", name=f"qt{m}")
                for m in range(2)
            ]
            kt_rot = [
                persist.tile([128, N], bf, tag=f"kt{m}", name=f"kt{m}")
                for m in range(2)
            ]
            # V for all heads with appended ones column: (128, NT, HL, 65)
            vones = persist.tile([128, NT, HL, DH + 1], bf, tag="vones", name="vones")
            nc.vector.memset(vones[:, :, :, DH : DH + 1], 1.0)
            # A^T head pairs: (128, N) bf16
            at2 = [
                persist.tile([128, N], bf, tag=f"at{m}", name=f"at{m}")
                for m in range(2)
            ]

            # ================= phase 1: projections =================
            with (
                tc.tile_pool(name="p1c", bufs=1) as p1c,
                tc.tile_pool(name="rot_tmp", bufs=3) as rot_tmp,
                tc.tile_pool(name="psum_qkt", bufs=2, space="PSUM") as psum_qkt,
                tc.tile_pool(name="psum_v", bufs=2, space="PSUM") as psum_v,
            ):
                wq_sb = p1c.tile([128, KC, DL], bf, tag="wq")
                wk_sb = p1c.tile([128, KC, DL], bf, tag="wk")
                wv_sb = p1c.tile([128, KC, DL], bf, tag="wv")
                ctab_sb = p1c.tile([128, 2, N], bf, tag="ctab")
                stab_sb = p1c.tile([128, 2, N], bf, tag="stab")
                xT_sb = p1c.tile([128, KC, N], bf, tag="xT")
                # order matters: the first QKT matmuls need wq + early xT
                # chunks; tables are only needed at the first eviction
                nc.sync.dma_start(out=wq_sb[:], in_=wq[:])
                for kc in range(KC):
                    nc.sync.dma_start(out=xT_sb[:, kc, :], in_=xT[:, kc, :])
                nc.sync.dma_start(out=wk_sb[:], in_=wk[:])
                nc.sync.dma_start(out=wv_sb[:], in_=wv[:])
                nc.sync.dma_start(out=ctab_sb[:], in_=ctab[:])
                nc.sync.dma_start(out=stab_sb[:], in_=stab[:])

                # Q^T / K^T: lhsT = w[kc, feats], rhs = xT[kc, toks]
                for w_sb, rot in ((wq_sb, qt_rot), (wk_sb, kt_rot)):
                    for mt in range(2):
                        for nh in range(NCH):  # 1024-wide tok chunks
                            nsl = slice(nh * 1024, (nh + 1) * 1024)
                            psum = psum_qkt.tile([128, 1024], f32, tag="qkt")
                            for kc in range(KC):
                                for nq in range(2):
                                    nc.tensor.matmul(
                                        psum[:, nq * 512 : (nq + 1) * 512],
                                        w_sb[:, kc, mt * 128 : (mt + 1) * 128],
                                        xT_sb[
                                            :,
                                            kc,
                                            nh * 1024
                                            + nq * 512 : nh * 1024
                                            + (nq + 1) * 512,
                                        ],
                                        start=(kc == 0),
                                        stop=(kc == KC - 1),
                                    )
                            # rotation eviction:
                            #   rot = raw * ctab + swap32(raw) * stab
                            raw = rot_tmp.tile([128, 1024], bf, tag="raw", name="raw")
                            nc.scalar.copy(out=raw[:], in_=psum[:])
                            nc.vector.tensor_mul(
                                rot[mt][:, nsl], raw[:], ctab_sb[:, mt, nsl]
                            )
                            raws = rot_tmp.tile([128, 1024], bf, tag="rs", name="raws")
                            for g in range(4):
                                s = g ^ 1
                                nc.vector.tensor_copy(
                                    raws[g * 32 : (g + 1) * 32, :],
                                    raw[s * 32 : (s + 1) * 32, :],
                                )
                            tmp = rot_tmp.tile([128, 1024], bf, tag="rt", name="tmp")
                            nc.vector.tensor_mul(tmp[:], raws[:], stab_sb[:, mt, nsl])
                            nc.vector.tensor_add(
                                rot[mt][:, nsl], rot[mt][:, nsl], tmp[:]
                            )

                # V: lhsT = xT[kc, toks], rhs = wv[kc, feats]
                for tt in range(NT):
                    vpsum = psum_v.tile([128, DL], f32, tag="v")
                    for kc in range(KC):
                        nc.tensor.matmul(
                            vpsum[:],
                            xT_sb[:, kc, tt * 128 : (tt + 1) * 128],
                            wv_sb[:, kc, :],
                            start=(kc == 0),
                            stop=(kc == KC - 1),
                        )
                    # single strided eviction for all 4 heads of this tile
                    nc.scalar.copy(
                        out=vones[:, tt, :, 0:DH],
                        in_=vpsum[:].rearrange("p (h d) -> p h d", h=HL),
                    )

            # ============ phase 2a: attention ============
            with (
                tc.tile_pool(name="p2c", bufs=1) as p2c,
                tc.tile_pool(name="exps_pool", bufs=5) as exps_pool,
                tc.tile_pool(name="norm_pool", bufs=2) as norm_pool,
                tc.tile_pool(name="bcast_pool", bufs=2) as bcast_pool,
                tc.tile_pool(name="dscr_pool", bufs=4, space="DRAM") as dscr_pool,
                tc.tile_pool(name="ostage_pool", bufs=3) as ostage_pool,
                tc.tile_pool(name="psum_s", bufs=3, space="PSUM") as psum_s,
                tc.tile_pool(name="psum_pv", bufs=3, space="PSUM") as psum_pv,
                tc.tile_pool(name="psum_o", bufs=2, space="PSUM") as psum_o,
            ):
                tri8_sb = p2c.tile([128, 8, 128], bf, tag="tri8")
                wo_sb = p2c.tile([128, 2, D], bf, tag="wo")
                nc.sync.dma_start(out=tri8_sb[:], in_=tri8[:])
                nc.sync.dma_start(out=wo_sb[:], in_=wo[:])

                def o_proj_block(qc):
                    # o-projection for the 4 token tiles whose A^T columns
                    # were normalized by 512-wide q-chunk qc
                    for tt in range(4 * qc, 4 * qc + 4):
                        ost = ostage_pool.tile([128, D], bf, tag="ost", name="ost")
                        for nb in range(2):
                            opsum = psum_o.tile([128, 512], f32, tag="o", name="opsum")
                            for hp in range(2):
                                nc.tensor.matmul(
                                    opsum[:],
                                    at2[hp][:, tt * 128 : (tt + 1) * 128],
                                    wo_sb[:, hp, nb * 512 : (nb + 1) * 512],
                                    start=(hp == 0),
                                    stop=(hp == 1),
                                )
                            if nb == 0:
                                nc.scalar.copy(
                                    out=ost[:, 0:512], in_=opsum[:]
                                )
                            else:
                                nc.vector.tensor_copy(
                                    out=ost[:, 512:D], in_=opsum[:]
                                )
                        nc.sync.dma_start(out=o_out[tt], in_=ost[:])

                dnm4s = {}

                def emit_s_exp(qc, mt):
                    # paired-head S matmuls: the two heads of pair `mt` sit
                    # in disjoint 64-partition halves of qt/kt tiles, so
                    # they run as independent row-tiles of the PE array
                    nkt = 4 * qc + 4
                    exps2 = [
                        exps_pool.tile([128, NT, 512], bf, tag="e", name="exps")
                        for _ in range(2)
                    ]
                    for kt in range(nkt):
                        j = kt - 4 * qc
                        jo = max(j, 0) * 128
                        for eo in range(2):
                            roff = eo * 64
                            spsum = psum_s.tile(
                                [128, 512], f32, tag="s", name="spsum"
                            )
                            nc.tensor.matmul(
                                spsum[:, jo:512],
                                kt_rot[mt][roff : roff + 64, kt * 128 : (kt + 1) * 128],
                                qt_rot[mt][
                                    roff : roff + 64,
                                    qc * 512 + jo : qc * 512 + 512,
                                ],
                                start=True,
                                stop=True,
                                tile_position=(roff, 0),
                            )
                            nc.scalar.activation(
                                exps2[eo][:, kt, jo:512], spsum[:, jo:512], AF.Exp
                            )
                    # mask the 4 diagonal 128x128 subtiles in one op each:
                    # element (p, j, c) -> exps[p, 4*qc + j, j*128 + c]
                    for eo in range(2):
                        sub = exps2[eo][:, 4 * qc, :]
                        diag = bass.AP(
                            tensor=sub.tensor,
                            offset=sub.offset,
                            ap=[list(sub.ap[0]), [640, 4], [1, 128]],
                        )
                        nc.vector.tensor_mul(diag, diag, tri8_sb[:, 0:4, :])
                    return exps2

                def emit_pv_evict(qc, mt, exps2):
                    qsl = slice(qc * 512, (qc + 1) * 512)
                    nkt = 4 * qc + 4
                    if mt == 0:
                        # denominator rows live at partitions 0/32/64/96
                        # (the only legal engine start partitions); unused
                        # rows are memset to 1.0 so the batched reciprocal
                        # stays finite
                        dnm4s[qc] = norm_pool.tile(
                            [97, 512], f32, tag="dnm", name="dnm4"
                        )
                        nc.gpsimd.memset(dnm4s[qc][:], 1.0)
                    dnm4 = dnm4s[qc]
                    for eo in range(2):
                        h = 2 * mt + eo
                        roff = eo * 64
                        # PV accumulation as one uninterrupted group
                        pv = psum_pv.tile([DH + 1, 512], f32, tag="pv", name="pv")
                        for kt in range(nkt):
                            j = kt - 4 * qc
                            jo = max(j, 0) * 128
                            nc.tensor.matmul(
                                pv[:, jo:512],
                                vones[:, kt, h, :],
                                exps2[eo][:, kt, jo:512],
                                start=(kt == 0),
                                stop=(kt == nkt - 1),
                            )
                        # stash denominator, evict unnormalized A^T
                        nc.vector.tensor_copy(
                            out=dnm4[32 * h : 32 * h + 1, :], in_=pv[DH : DH + 1, :]
                        )
                        nc.vector.tensor_copy(
                            out=at2[mt][roff : roff + DH, qsl], in_=pv[0:DH, :]
                        )

                def emit_normalize(qc):
                    qsl = slice(qc * 512, (qc + 1) * 512)
                    rcp4 = norm_pool.tile([97, 512], f32, tag="rcp", name="rcp4")
                    nc.vector.reciprocal(rcp4[:], dnm4s[qc][:])
                    for mt in range(2):
                        bc = bcast_pool.tile([128, 512], f32, tag="bc", name="bc")
                        # broadcast each head's reciprocal row across 64
                        # partitions: bounce through DRAM, then a step-0
                        # partition DMA (legal for DRAM sources only; POOL's
                        # partition_broadcast ignores non-zero base
                        # partitions on hardware)
                        for half in range(2):
                            row = rcp4[64 * mt + 32 * half : 64 * mt + 32 * half + 1, :]
                            rdram = dscr_pool.tile([1, 512], f32, tag="rd", name="rd")
                            nc.sync.dma_start(out=rdram[:], in_=row)
                            rd = rdram[:]
                            nc.sync.dma_start(
                                out=bc[64 * half : 64 * half + 64, :],
                                in_=bass.AP(
                                    tensor=rd.tensor,
                                    offset=rd.offset,
                                    ap=[[0, 64], [1, 512]],
                                ),
                            )
                        nc.vector.tensor_mul(at2[mt][:, qsl], at2[mt][:, qsl], bc[:])

                # software-pipelined emission: S/exp of iteration i, then PV
                # of iteration i-1, so PV never waits on a fresh exp tail
                iters = [(qc, mt) for qc in range(4) for mt in range(2)]
                pending = None  # (qc, mt, exps2)
                for qc, mt in iters:
                    exps2 = emit_s_exp(qc, mt)
                    if pending is not None:
                        emit_pv_evict(*pending)
                        if pending[1] == 1:
                            emit_normalize(pending[0])
                            o_proj_block(pending[0])
                    pending = (qc, mt, exps2)
                emit_pv_evict(*pending)
                emit_normalize(pending[0])
                o_proj_block(pending[0])

    nc.compile()
    return nc


def get_program():
    global _PROG
    if _PROG is None:
        _PROG = _build_program()
    return _PROG


def _host_tables(bit_logits):
    """Replicate the reference fp32 cos/sin computation exactly (jax on CPU)."""
    import jax

    with jax.default_device(jax.devices("cpu")[0]):
        import jax.numpy as jnp

        basis = jnp.asarray(EULER_BASIS, dtype=jnp.float32)
        freqs = jax.nn.sigmoid(jnp.asarray(bit_logits, dtype=jnp.float32)) @ basis
        inv_freq = 2.0 ** (-(jnp.arange(0, DH, 2, dtype=jnp.float32) / DH))
        pos = jnp.arange(N, dtype=jnp.float32)
        theta = pos[None, :, None] * freqs[:, None, None] * inv_freq[None, None, :]
        cos = np.asarray(jnp.cos(theta))  # (H, N, 32)
        sin = np.asarray(jnp.sin(theta))
    return cos, sin


def _chunk_rows(a, p=128):
    """(R, C) -> (p, R//p, C); row r = kc*p + pp lands at [pp, kc]."""
    r, c = a.shape
    return np.ascontiguousarray(a.reshape(r // p, p, c).transpose(1, 0, 2))


def prepare_inputs(x, w_qkv, w_o, bit_logits):
    import ml_dtypes

    bf = ml_dtypes.bfloat16

    x = np.asarray(x, dtype=np.float32)
    w_qkv = np.asarray(w_qkv, dtype=np.float32)
    w_o = np.asarray(w_o, dtype=np.float32)
    cos, sin = _host_tables(np.asarray(bit_logits, dtype=np.float32))

    # de-interleave permutation within a head: evens then odds
    perm = np.concatenate([np.arange(0, DH, 2), np.arange(1, DH, 2)])

    wq_full = w_qkv.reshape(D, 3, H, DH)[:, 0]  # (D, H, DH)
    wk_full = w_qkv.reshape(D, 3, H, DH)[:, 1]
    wv_full = w_qkv.reshape(D, 3, H, DH)[:, 2]
    scale = 1.0 / math.sqrt(DH)

    # tri[krow, qcol] = 1 if qcol >= krow else 0, replicated 8x for the
    # strided diagonal mask
    tri = np.triu(np.ones((128, 128), dtype=np.float32))
    tri8 = np.broadcast_to(tri[:, None, :], (128, 8, 128)).copy()

    xT_by_batch = [
        _chunk_rows(np.ascontiguousarray(x[b].T)) for b in range(B)
    ]  # (128, KC, N)

    per_group = []
    for g in range(4):
        heads = range(4 * g, 4 * g + 4)
        wq_g = np.concatenate(
            [wq_full[:, h][:, perm] * scale for h in heads], axis=1
        )  # (D, 256)
        wk_g = np.concatenate([wk_full[:, h][:, perm] for h in heads], axis=1)
        wv_g = np.concatenate([wv_full[:, h] for h in heads], axis=1)
        wo_g = np.concatenate(
            [w_o.reshape(H, DH, D)[h] for h in heads], axis=0
        )  # (256, D)

        # rotation tables, layout (256 feats, N) -> (128, 2, N)
        ct = np.empty((DL, N), dtype=np.float32)
        st = np.empty((DL, N), dtype=np.float32)
        for hl, h in enumerate(heads):
            c = cos[h].T  # (32, N)
            s = sin[h].T
            ct[hl * DH : hl * DH + 32] = c
            ct[hl * DH + 32 : hl * DH + 64] = c
            st[hl * DH : hl * DH + 32] = -s
            st[hl * DH + 32 : hl * DH + 64] = s
        per_group.append(
            dict(
                wq=_chunk_rows(wq_g).astype(bf),
                wk=_chunk_rows(wk_g).astype(bf),
                wv=_chunk_rows(wv_g).astype(bf),
                wo=_chunk_rows(wo_g).astype(bf),
                ctab=_chunk_rows(ct).astype(bf),
                stab=_chunk_rows(st).astype(bf),
                tri8=tri8.astype(bf),
            )
        )

    in_maps = []
    for c in range(NCORES):
        b, g = c // 4, c % 4
        m = dict(per_group[g])
        m["xT"] = xT_by_batch[b].astype(bf)
        in_maps.append(m)
    return in_maps


def kernel(x, w_qkv, w_o, bit_logits, n_heads):
    global LAST_RESULTS
    from concourse.bass_utils import run_bass_kernel_spmd

    assert int(n_heads) == H
    nc = get_program()
    in_maps = prepare_inputs(x, w_qkv, w_o, bit_logits)
    res = run_bass_kernel_spmd(nc, in_maps, list(range(NCORES)))
    LAST_RESULTS = res
    out = np.zeros((B, N, D), dtype=np.float32)
    for c in range(NCORES):
        b = c // 4
        out[b] += res.results[c]["o_out"].reshape(N, D).astype(np.float32)
    return out
